# revision 17
# baseline (speedup 1.0000x reference)
"""Trainium2 Bass kernel for nn_AttentionBlock (GroupNorm + 1x1-conv QKV +
softmax attention + 1x1-conv proj + residual), B=4 C=512 H=W=64 HEADS=8.

Sharding: 8 cores = (batch b in 0..4) x (query-half ih in 0..2).  Each core
computes groupnorm + K/V for its whole batch (duplicated across the 2 cores
sharing a batch -- cheap) and attention + proj for its 2048 queries.  Cores
are fully independent SPMD (no collectives); the host splits and concats.
For ih=1 cores the host *rolls* the spatial columns of x by 2048 so that the
query half is always columns [0:2048).

Key optimizations over the bf16 baseline:
- The softmax exp (67M elems/core) is split across ScalarE (LUT exp,
  fp8e4 out) and DVE (Schraudolph exp: one tensor_scalar f32->int8 with
  truncation, bit-cast as fp8e4m3).  P stays fp8 everywhere.
- O = P @ V_ext runs in fp8 DoubleRow mode (contraction 256 = two key
  tiles per instruction) -- 2x fewer PE columns streamed than bf16.
- QKV and proj matmuls also fp8 DoubleRow (weights pre-scaled x256 on the
  host to dodge fp8 subnormals; the 1/256 is folded into the PSUM evac).
- Per-(head-pair, query-block) attention: scores bf16 with 2-head PE row
  packing (auto tile_position row tiling); jt-pair software pipeline feeds
  DoubleRow O one step behind.
- Softmax denominator: O is evacuated UNNORMALIZED (x 1/16, fp8); the
  colsum rows are gathered into an [8, 2048] SBUF tile, inverted once per
  head-pair with a single DVE reciprocal_approx_fast, broadcast to 128
  partitions via a DRAM-roundtrip DMA, and applied with one batched
  [128, 2048] DVE multiply per head-pair.  This removes all Ln/Exp
  activations from the attention loop, which kills the ACT table-set
  thrashing (natural_log vs exp_and_others reload = 1.3us each) that the
  old per-block exp(-ln(cs)) reciprocal caused.
- sc PSUM tiles triple-buffered (the exp(jt-3) -> scores MM(jt) -> exp(jt)
  recycle loop must be longer than one exp); block n's o_ps evacuation is
  deferred into the first jts of block n+1.
- GroupNorm rstd = exp(-0.5*ln(var+eps)) is batched into one Ln + one Exp
  over all 4 channel tiles (2 table loads total, in the prologue).
- GPSIMD (no PSUM access) takes SBUF->SBUF work: groupnorm normalize,
  proj residual add.  QKV PSUM evacuations are split ACT/DVE.
- x is DMA'd once and stays resident in SBUF (stats + normalize + residual).
"""

from contextlib import ExitStack

import numpy as np
import ml_dtypes

import concourse.bass as bass
import concourse.tile as tile
import concourse.mybir as mybir
from concourse import bacc
from concourse.bass_utils import run_bass_kernel_spmd

F32 = mybir.dt.float32
BF16 = mybir.dt.bfloat16
F8 = mybir.dt.float8e4
I8 = mybir.dt.int8
EXP = mybir.ActivationFunctionType.Exp
IDENT = mybir.ActivationFunctionType.Identity
LN = mybir.ActivationFunctionType.Ln
MULT = mybir.AluOpType.mult
ADD = mybir.AluOpType.add
DR = mybir.MatmulPerfMode.DoubleRow

B, C, HH, WW = 4, 512, 64, 64
S = HH * WW              # 4096
HEADS = 8
HD = C // HEADS          # 64
GROUPS = 32
GSIZE = C // GROUPS      # 16 channels per group
EPS = 1e-5
SCALE = 1.0 / 8.0        # 1/sqrt(head_dim)
IHALF = S // 2           # 2048 queries per core
CT = C // 128            # 4 channel tiles
ST = S // 128            # 32 spatial (key) tiles
WSCALE = 256.0           # host-side fp8 weight pre-scale
OSCALE = 1.0 / 16.0      # unnormalized-O evac scale (fp8 headroom)
IWS = 512                # query block
NIW = IHALF // IWS       # 4 query blocks

# Schraudolph exp2 constants for fp8e4m3 bit pattern (HW f32->int8 convert
# rounds to nearest): bits8 = round(s * A8 + B8);  A8 = 2^3*log2(e)*SCALE
A8 = 8.0 * 1.4426950408889634 * SCALE
B8 = 56.0 - 0.42

# per-jt exp engine: 'A' = ScalarE LUT exp, 'D' = DVE Schraudolph
def _mk_pat(counts):
    tot = sum(counts.values())
    acc = {k: 0.0 for k in counts}
    out = []
    for _ in range(tot):
        for k in counts:
            acc[k] += counts[k] / tot
        k = max(acc, key=lambda q: acc[q])
        acc[k] -= 1.0
        out.append(k)
    return "".join(out)


EXP_PAT = _mk_pat({"A": 18, "D": 14})
assert len(EXP_PAT) == ST


import os
DBG = bool(os.environ.get("K_DBG"))


def build_kernel(reps: int = 1):
    nc = bacc.Bacc("TRN2", target_bir_lowering=False, debug=False)

    x_d = nc.dram_tensor("x", [C, S], F32, kind="ExternalInput").ap()
    x8_d = nc.dram_tensor("x8", [C, S], F8, kind="ExternalInput").ap()
    wn_d = nc.dram_tensor("wn", [4, GROUPS, C], BF16, kind="ExternalInput").ap()
    beta_dram = nc.dram_tensor("beta_scratch", [GROUPS, 1], BF16, kind="Internal").ap()
    qw_d = nc.dram_tensor("qw8", [C, C], F8, kind="ExternalInput").ap()   # (qkv_w[0:512].T * 256)
    kw_d = nc.dram_tensor("kw8", [C, C], F8, kind="ExternalInput").ap()
    vw_d = nc.dram_tensor("vw8", [C, C], F8, kind="ExternalInput").ap()
    pw_d = nc.dram_tensor("pw8", [C, C], F8, kind="ExternalInput").ap()   # proj_w.T * 256
    bias_d = nc.dram_tensor("biases", [4, C], F32, kind="ExternalInput").ap()  # qb,kb,vb,pb
    nwb_d = nc.dram_tensor("nwb", [2, C], F32, kind="ExternalInput").ap()      # norm_w, norm_b
    m8_d = nc.dram_tensor("mask8", [128, 8], F32, kind="ExternalInput").ap()
    m8t_d = nc.dram_tensor("mask8t", [8, 128], F32, kind="ExternalInput").ap()
    cs_dram = nc.dram_tensor("cs_scratch", [8, IHALF], F32, kind="Internal").ap()
    out_d = nc.dram_tensor("out", [C, IHALF], F32, kind="ExternalOutput").ap()
    dbg = {}
    if DBG:
        dbg["h"] = nc.dram_tensor("dbg_h", [128, CT, S], F8, kind="ExternalOutput").ap()
        dbg["k"] = nc.dram_tensor("dbg_k", [128, CT, S], BF16, kind="ExternalOutput").ap()
        dbg["q"] = nc.dram_tensor("dbg_q", [128, CT, IHALF], BF16, kind="ExternalOutput").ap()
        dbg["vt"] = nc.dram_tensor("dbg_vt", [128, ST, HEADS, 66], F8, kind="ExternalOutput").ap()
        dbg["ex"] = nc.dram_tensor("dbg_ex", [128, 2, 2, 2, IWS], F8, kind="ExternalOutput").ap()
        dbg["on"] = nc.dram_tensor("dbg_on", [128, CT, IHALF], F8, kind="ExternalOutput").ap()
        dbg["cs"] = nc.dram_tensor("dbg_cs", [4, 2, IHALF], F32, kind="ExternalOutput").ap()

    with tile.TileContext(nc) as tc:
        with ExitStack() as ctx:
            const = ctx.enter_context(tc.tile_pool(name="const", bufs=1))

            wts = {}
            for nm, ap in [("qw", qw_d), ("kw", kw_d), ("vw", vw_d), ("pw", pw_d)]:
                t8 = const.tile([128, CT, C], F8, name=f"{nm}_f8")
                nc.sync.dma_start(out=t8, in_=ap.rearrange("(kt p) c -> p kt c", p=128))
                wts[nm] = t8

            bias_cols = const.tile([128, 4, CT], F32)
            nc.sync.dma_start(out=bias_cols,
                              in_=bias_d.rearrange("w (kt p) -> p w kt", p=128))
            nwb_t = const.tile([128, 2, CT], F32)
            nc.sync.dma_start(out=nwb_t, in_=nwb_d.rearrange("w (kt p) -> p w kt", p=128))

            wn_t = const.tile([GROUPS, 4, CT, 128], BF16)
            nc.sync.dma_start(out=wn_t,
                              in_=wn_d.rearrange("w g (mt p) -> g w mt p", p=128))

            m8_t = const.tile([128, 8], F32)
            nc.sync.dma_start(out=m8_t, in_=m8_d)
            m8t_t = const.tile([8, 128], F32)
            nc.sync.dma_start(out=m8t_t, in_=m8t_d)

            eps_t = const.tile([8, 1], F32)
            nc.vector.memset(eps_t, EPS)

            consts = (wts, wn_t, bias_cols, nwb_t, m8_t, m8t_t, eps_t)
            if reps == 1:
                _one_pass(nc, tc, x_d, x8_d, out_d, cs_dram, beta_dram, consts, dbg)
            else:
                with tc.For_i(0, reps, 1):
                    _one_pass(nc, tc, x_d, x8_d, out_d, cs_dram, beta_dram, consts, dbg)
    nc.compile()
    return nc


def _one_pass(nc, tc, x_d, x8_d, out_d, cs_dram, beta_dram, consts, dbg={}):
    (wts, wn_t, bias_cols, nwb_t, m8_t, m8t_t, eps_t) = consts
    with ExitStack() as ps:
        big = ps.enter_context(tc.tile_pool(name="big", bufs=1))
        x8_all = big.tile([128, CT, S], F8)             # 16 KB (host fp8 cast)
        wsc = {nm: big.tile([128, CT, C], F8, name=f"{nm}_sc")
               for nm in ("qw", "kw", "vw")}            # affine-scaled weights
        bias_eff = big.tile([128, 4, CT], F32)          # q/k/v/proj biases after absorb
        k_all = big.tile([128, CT, S], BF16)            # 32 KB
        q_all = big.tile([128, CT, IHALF], BF16)        # 16 KB
        vt_ext = big.tile([128, ST, HEADS, 66], F8)     # ~16.5 KB
        o_raw8 = big.tile([128, CT, IHALF], F8)         # 8 KB  (unnormalized o/16)
        o_norm = big.tile([128, CT, IHALF], F8)         # 8 KB
        ex_all = big.tile([128, 2, 2, 2, IWS], F8)      # (head, jtpbuf, t, cols) 2 KB

        svec = ps.enter_context(tc.tile_pool(name="svec", bufs=1))
        aff_s = svec.tile([128, CT], F32)
        aff_t = svec.tile([128, CT], F32)
        # colsums/16 for (hp, p) at partition 32*hp, free dim p (32-aligned
        # partition bases are required for engine operands); the reciprocal
        # runs in place per head-pair.
        cs_ab = svec.tile([128, 2, IHALF], F32)

        # =============== P1: x8 load + groupnorm statistics ===============
        # Stats run on the host-cast fp8 x (stat noise ~1e-3 rel, well inside
        # budget) so the 8MB f32 x never enters SBUF.  The residual is
        # instead pre-copied DRAM->DRAM into out and the proj result is
        # DMA-accumulated onto it at the end.
        for t in range(CT):
            eng = nc.sync if t % 2 == 0 else nc.scalar
            eng.dma_start(out=x8_all[:, t, :], in_=x8_d[t * 128:(t + 1) * 128, :])
        xc = nc.scalar.dma_start(out=out_d, in_=x_d[:, 0:IHALF])
        tc.chain_iter_dep("xres", xc.ins)

        with tc.tile_pool(name="stat", bufs=2) as stat, \
             tc.tile_pool(name="gn_ps", bufs=1, space="PSUM") as gn_ps:
            gstats = gn_ps.tile([8, CT, 2], F32)
            warm_ps = gn_ps.tile([6, 512], F32)
            warm_src = stat.tile([128, 6], F8, tag="wsrc", bufs=1)
            for t in range(CT):
                bnst = stat.tile([128, 8, 6], F32, tag="bnst")
                for sg in range(8):
                    nc.vector.bn_stats(out=bnst[:, sg, :],
                                       in_=x8_all[:, t, sg * 512:(sg + 1) * 512])
                mv = stat.tile([128, 2], F32, tag="mv")
                nc.vector.bn_aggr(out=mv, in_=bnst)
                st_t = stat.tile([128, 2], F32, tag="st")   # (mean, 2nd moment)
                nc.vector.tensor_copy(st_t[:, 0:1], mv[:, 0:1])
                sq = stat.tile([128, 1], F32, tag="sq")
                nc.vector.tensor_mul(sq, mv[:, 0:1], mv[:, 0:1])
                nc.vector.tensor_add(st_t[:, 1:2], mv[:, 1:2], sq)
                nc.tensor.matmul(gstats[:, t, :], m8_t, st_t, start=True, stop=True)
                if t == 0:
                    nc.vector.tensor_copy(warm_src, bnst[:, 0, :])
            # PE keep-warm dummies after the gstats chain (bridge to qkv)
            for _ in range(2):
                nc.tensor.matmul(warm_ps, warm_src, x8_all[:, 0, 0:512],
                                 start=True, stop=True)

            gs = stat.tile([8, CT, 2], F32, tag="gs")
            nc.vector.tensor_copy(gs, gstats)
            grp = stat.tile([8, CT, 2], F32, tag="grp")   # (mu_g, rstd_g)
            var_all = stat.tile([8, CT], F32, tag="var")
            for t in range(CT):
                nc.scalar.mul(grp[:, t, 0:1], gs[:, t, 0:1], 1.0 / GSIZE)
                e_t = stat.tile([8, 1], F32, tag="e")
                nc.scalar.mul(e_t, gs[:, t, 1:2], 1.0 / GSIZE)
                musq = stat.tile([8, 1], F32, tag="musq")
                nc.vector.tensor_mul(musq, grp[:, t, 0:1], grp[:, t, 0:1])
                nc.vector.tensor_sub(var_all[:, t:t + 1], e_t, musq)
            # rstd = exp(-0.5*ln(var+eps)), batched over all CT tiles:
            # one Ln + one Exp = exactly two ACT table loads, in the prologue.
            lnv = stat.tile([8, CT], F32, tag="lnv")
            nc.scalar.activation(lnv, var_all, LN, bias=eps_t)
            nc.scalar.activation(grp[:, :, 1], lnv, EXP, scale=-0.5)

            bc_ps = gn_ps.tile([128, CT, 2], F32)
            for t in range(CT):
                nc.tensor.matmul(bc_ps[:, t, :], m8t_t, grp[:, t, :], start=True, stop=True)
            bc = stat.tile([128, CT, 2], F32, tag="bc")
            nc.vector.tensor_copy(bc, bc_ps)
            nc.vector.tensor_mul(aff_s, nwb_t[:, 0, :], bc[:, :, 1])
            # beta_g = mu_g * rstd_g -> [32, 1] via DRAM roundtrip (engines
            # cannot re-map partitions; DMA can)
            beta8 = stat.tile([8, CT], BF16, tag="beta8")
            nc.vector.tensor_mul(beta8, grp[:, :, 0], grp[:, :, 1])
            wb = nc.sync.dma_start(out=beta_dram.rearrange("(kt i) o -> i (kt o)", i=8),
                                   in_=beta8)
            tc.chain_iter_dep("betadram", wb.ins)
            beta32 = stat.tile([GROUPS, 1], BF16, tag="beta32")
            rb_ = nc.sync.dma_start(out=beta32, in_=beta_dram)
            tc.chain_iter_dep("betadram", rb_.ins)

        # =============== P2: absorb groupnorm affine into qkv weights ======
        # w'[o,c] = w[o,c] * s[c] (per-partition scale of the stationary
        # side); bias' = b + W@norm_b (host) - Wn @ beta (12 tiny f32 MMs).
        for i, nm in enumerate(("qw", "kw", "vw")):
            for kt in range(CT):
                if (i + kt) % 2 == 0:
                    nc.scalar.activation(wsc[nm][:, kt, :], wts[nm][:, kt, :],
                                         IDENT, scale=aff_s[:, kt:kt + 1])
                else:
                    nc.vector.tensor_scalar(
                        out=wsc[nm][:, kt, :], in0=wts[nm][:, kt, :],
                        scalar1=aff_s[:, kt:kt + 1], scalar2=None, op0=MULT)
        with tc.tile_pool(name="bias_ps", bufs=1, space="PSUM") as bias_ps:
            bps = bias_ps.tile([128, 4, CT], F32)
            for w in range(4):
                for mt in range(CT):
                    nc.tensor.matmul(bps[:, w, mt:mt + 1], wn_t[:, w, mt, :],
                                     beta32, start=True, stop=True)
            nc.vector.tensor_sub(bias_eff, bias_cols, bps)

        nc.vector.memset(vt_ext[:, :, :, 64:66], 0.0)
        nc.vector.memset(vt_ext[:, :, :, 64:65], 1.0)

        # =============== P3: qkv projections (fp8 DoubleRow) ===============
        with tc.tile_pool(name="qkv_ps", bufs=3, space="PSUM") as qkv_ps:
            for dst, w8, bidx, ncols in ((k_all, wsc["kw"], 1, S),
                                         (q_all, wsc["qw"], 0, IHALF)):
                for mt in range(CT):
                    for ic2 in range(ncols // 1024):
                        acc = qkv_ps.tile([128, 1024], F32, tag="qkps")
                        for j in range(2):
                            icol = 2 * ic2 + j
                            for kp in range(2):
                                nc.tensor.matmul(
                                    acc[:, j * 512:(j + 1) * 512],
                                    w8[:, 2 * kp:2 * kp + 2, mt * 128:(mt + 1) * 128],
                                    x8_all[:, 2 * kp:2 * kp + 2, icol * 512:(icol + 1) * 512],
                                    start=(kp == 0), stop=(kp == 1), perf_mode=DR)
                        # one 1024-wide evac per pair, split ACT/DVE
                        if (mt + ic2) % 2 == 0:
                            nc.scalar.activation(
                                dst[:, mt, ic2 * 1024:(ic2 + 1) * 1024], acc, IDENT,
                                scale=1.0 / WSCALE, bias=bias_eff[:, bidx, mt:mt + 1])
                        else:
                            nc.vector.tensor_scalar(
                                out=dst[:, mt, ic2 * 1024:(ic2 + 1) * 1024], in0=acc,
                                scalar1=1.0 / WSCALE, scalar2=bias_eff[:, bidx, mt:mt + 1],
                                op0=MULT, op1=ADD)
            # V^T: [S, 512c] scattered into vt_ext (64 cols per head + ones
            # col).  The V bias is folded into the proj bias on the host
            # (pb' = pb + proj_w @ vb), so the evac is a pure scale and can
            # split across ACT and DVE.
            for sp in range(ST // 2):
                acc = qkv_ps.tile([128, 1024], F32, tag="qkps")
                for j in range(2):
                    st = 2 * sp + j
                    for kp in range(2):
                        nc.tensor.matmul(
                            acc[:, j * 512:(j + 1) * 512],
                            x8_all[:, 2 * kp:2 * kp + 2, st * 128:(st + 1) * 128],
                            wsc["vw"][:, 2 * kp:2 * kp + 2, :],
                            start=(kp == 0), stop=(kp == 1), perf_mode=DR)
                if sp % 2 == 0:
                    nc.scalar.activation(
                        vt_ext[:, 2 * sp:2 * sp + 2, :, 0:64],
                        acc.rearrange("p (s h d) -> p s h d", s=2, d=HD), IDENT,
                        scale=1.0 / WSCALE)
                else:
                    nc.vector.tensor_scalar(
                        out=vt_ext[:, 2 * sp:2 * sp + 2, :, 0:64],
                        in0=acc.rearrange("p (s h d) -> p s h d", s=2, d=HD),
                        scalar1=1.0 / WSCALE, scalar2=None, op0=MULT)

        if dbg:
            nc.sync.dma_start(out=dbg["h"], in_=x8_all)
            nc.sync.dma_start(out=dbg["k"], in_=k_all)
            nc.sync.dma_start(out=dbg["q"], in_=q_all)
            nc.sync.dma_start(out=dbg["vt"], in_=vt_ext)

        # =============== P4: attention ===============
        # Per (head-pair hp, query block iw of 512): scores bf16 with 2-head
        # row packing -> PSUM [128, 2, 512]; exp on ScalarE/DVE per EXP_PAT
        # -> ex fp8; O accumulates per head in fp8 DoubleRow over jt pairs,
        # pipelined one jt-pair behind the scores/exp.  o_ps double-buffered
        # across blocks; evacuation of block n is issued early in block n+1.
        with tc.tile_pool(name="att_ps", bufs=1, space="PSUM") as att_ps, \
             tc.tile_pool(name="att_sb", bufs=1) as att_sb:
            o_ps = [att_ps.tile([65, IWS], F32, tag=f"o{p}", name=f"o_ps{p}")
                    for p in range(2)]
            rb_t = att_sb.tile([128, IHALF], F32, name="rb")

            blocks = [(hp, iw) for hp in range(CT) for iw in range(NIW)]
            deferred = {"A": [], "D": []}   # evac closures from previous block
            chain = {}                      # jt -> closure (recip / dma / mult)

            def mk_evac(hp, iw, o_ps):
                # evacuate unnormalized o (x 1/16) + gather colsum rows
                def evac_a():
                    nc.scalar.activation(
                        o_raw8[0:64, hp, iw * IWS:(iw + 1) * IWS],
                        o_ps[0][0:64, :], IDENT, scale=OSCALE)
                    nc.scalar.activation(
                        cs_ab[32 * hp:32 * hp + 1, 0, iw * IWS:(iw + 1) * IWS],
                        o_ps[0][64:65, :], IDENT, scale=OSCALE)
                def evac_d():
                    nc.vector.tensor_scalar(
                        out=o_raw8[64:128, hp, iw * IWS:(iw + 1) * IWS],
                        in0=o_ps[1][0:64, :], scalar1=OSCALE, scalar2=None, op0=MULT)
                    nc.vector.tensor_scalar(
                        out=cs_ab[32 * hp:32 * hp + 1, 1, iw * IWS:(iw + 1) * IWS],
                        in0=o_ps[1][64:65, :], scalar1=OSCALE, scalar2=None, op0=MULT)
                return evac_a, evac_d

            def mk_chain(hp, rb):
                # 16/cs for head-pair hp: DRAM-roundtrip broadcast of the raw
                # colsums to 128 partitions, then one in-place DVE
                # reciprocal_approx_fast on the broadcast tile (base partition
                # 0 -- the custom-DVE op mis-addresses nonzero base
                # partitions; engine cost is free-size only, so recip on
                # [128, 2048] costs the same as on [1, 2048]), then one
                # batched normalize multiply.
                def c_dma():
                    w = nc.sync.dma_start(out=cs_dram[2 * hp:2 * hp + 2, :],
                                          in_=cs_ab[32 * hp:32 * hp + 1, :, :])
                    tc.chain_iter_dep("csdram", w.ins)
                    b0 = nc.sync.dma_start(
                        out=rb[0:64, :],
                        in_=cs_dram[2 * hp:2 * hp + 1, :].to_broadcast([64, IHALF]))
                    tc.chain_iter_dep("csdram", b0.ins)
                    b1 = nc.sync.dma_start(
                        out=rb[64:128, :],
                        in_=cs_dram[2 * hp + 1:2 * hp + 2, :].to_broadcast([64, IHALF]))
                    tc.chain_iter_dep("csdram", b1.ins)
                def c_recip():
                    nc.vector.reciprocal_approx_fast(out=rb, in_=rb)
                def c_mult():
                    nc.vector.tensor_mul(o_norm[:, hp, :], o_raw8[:, hp, :], rb)
                return [c_dma, c_recip, c_mult]

            for bi, (hp, iw) in enumerate(blocks):
                def emit_o(jtp, p):
                    nc.tensor.matmul(
                        o_ps[p],
                        vt_ext[:, 2 * jtp:2 * jtp + 2, 2 * hp + p, 0:65],
                        ex_all[:, p, jtp % 2, :, :],
                        start=(jtp == 0), stop=(jtp == ST // 2 - 1),
                        perf_mode=DR, skip_group_check=True)

                for jtp in range(ST // 2):
                    for t in range(2):
                        jt = 2 * jtp + t
                        sc = att_ps.tile([128, 2, IWS], F32, tag=f"sc{jt % 3}",
                                         name=f"sc{jt % 3}")
                        for p in range(2):
                            nc.tensor.matmul(
                                sc[:, p, :],
                                k_all[64 * p:64 * p + 64, hp, jt * 128:(jt + 1) * 128],
                                q_all[64 * p:64 * p + 64, hp, iw * IWS:(iw + 1) * IWS],
                                start=True, stop=True)
                        ex_ap = ex_all[:, :, jtp % 2, t, :]
                        if EXP_PAT[jt] == "A":
                            nc.scalar.activation(ex_ap, sc, EXP, scale=SCALE)
                        else:
                            nc.vector.tensor_scalar(
                                out=ex_ap.bitcast(I8), in0=sc,
                                scalar1=A8, scalar2=B8, op0=MULT, op1=ADD)
                        # flush previous block's evacuations / recip chain
                        if jt == 1 and deferred["A"]:
                            for f in deferred["A"]:
                                f()
                            deferred["A"] = []
                        elif jt == 2 and deferred["D"]:
                            for f in deferred["D"]:
                                f()
                            deferred["D"] = []
                        elif jt in chain:
                            chain.pop(jt)()
                    if jtp > 0:
                        for p in range(2):
                            emit_o(jtp - 1, p)
                    if dbg and hp == 0 and iw == 0 and jtp == 0:
                        nc.sync.dma_start(out=dbg["ex"], in_=ex_all)
                for p in range(2):
                    emit_o(ST // 2 - 1, p)

                evac_a, evac_d = mk_evac(hp, iw, o_ps)
                if bi == len(blocks) - 1:
                    evac_a(); evac_d()
                    for f in mk_chain(hp, rb_t):
                        f()
                else:
                    deferred["A"].append(evac_a)
                    deferred["D"].append(evac_d)
                    if iw == NIW - 1:
                        # schedule hp's normalize chain into upcoming blocks
                        fns = mk_chain(hp, rb_t)
                        chain[6] = fns[0]    # dma    (block bi+1, jt 6)
                        chain[18] = fns[1]   # recip  (block bi+1, jt 18)
                        chain[20] = fns[2]   # mult   (block bi+1, jt 20)


        if dbg:
            nc.sync.dma_start(out=dbg["on"], in_=o_norm)
            for hp in range(CT):
                nc.sync.dma_start(out=dbg["cs"][hp], in_=cs_ab[32 * hp:32 * hp + 1, :, :])

        # =============== P5: proj + residual ===============
        # out already holds x (DRAM precopy); evac applies scale+bias, then a
        # gpsimd SWDGE DMA accumulates the proj result onto out (one per mt).
        with tc.tile_pool(name="pj_ps", bufs=3, space="PSUM") as pj_ps, \
             tc.tile_pool(name="pj_sb", bufs=2) as pj_sb:
            for mt in range(CT):
                o_mt = pj_sb.tile([128, IHALF], F32, tag="ot")
                for icol in range(IHALF // 512):
                    acc = pj_ps.tile([128, 512], F32, tag="pj")
                    for kp in range(2):
                        nc.tensor.matmul(
                            acc, wts["pw"][:, 2 * kp:2 * kp + 2, mt * 128:(mt + 1) * 128],
                            o_norm[:, 2 * kp:2 * kp + 2, icol * 512:(icol + 1) * 512],
                            start=(kp == 0), stop=(kp == 1), perf_mode=DR)
                    if icol % 2 == 0:
                        nc.scalar.activation(
                            o_mt[:, icol * 512:(icol + 1) * 512], acc, IDENT,
                            scale=1.0 / WSCALE, bias=bias_eff[:, 3, mt:mt + 1])
                    else:
                        nc.vector.tensor_scalar(
                            out=o_mt[:, icol * 512:(icol + 1) * 512], in0=acc,
                            scalar1=1.0 / WSCALE, scalar2=bias_eff[:, 3, mt:mt + 1],
                            op0=MULT, op1=ADD)
                ac = nc.gpsimd.dma_start(
                    out=out_d[mt * 128:(mt + 1) * 128, :], in_=o_mt,
                    accum_op=ADD)
                tc.chain_iter_dep("xres", ac.ins)


# ---------------------------------------------------------------------------
# host side
# ---------------------------------------------------------------------------

_CACHE = {}


def _get_nc(reps=1):
    if reps not in _CACHE:
        _CACHE[reps] = build_kernel(reps)
    return _CACHE[reps]


def _w8(a):
    return np.ascontiguousarray((a * WSCALE).astype(ml_dtypes.float8_e4m3))


def _make_in_maps(inputs):
    x = np.ascontiguousarray(np.asarray(inputs["x"], dtype=np.float32))
    qkv_w = np.asarray(inputs["qkv_w"], dtype=np.float32)
    qkv_b = np.asarray(inputs["qkv_b"], dtype=np.float32)
    proj_w = np.asarray(inputs["proj_w"], dtype=np.float32)
    proj_b = np.asarray(inputs["proj_b"], dtype=np.float32)
    norm_w = np.asarray(inputs["norm_w"], dtype=np.float32)
    norm_b = np.asarray(inputs["norm_b"], dtype=np.float32)

    wq, wk, wv = qkv_w[0:C], qkv_w[C:2 * C], qkv_w[2 * C:3 * C]
    # group-collapsed weights (for the on-device bias correction -Wn@beta)
    wn3 = [(w * norm_w[None, :]).reshape(C, GROUPS, GSIZE).sum(axis=2)
           for w in (wq, wk, wv)]                      # each [512(out), 32(g)]
    wn = np.stack([wn3[0].T, wn3[1].T, wn3[2].T,
                   (proj_w @ wn3[2]).T])               # [4, 32, 512]
    vb_full = qkv_b[2 * C:3 * C] + wv @ norm_b   # v-bias base (beta part on dev)
    shared = {
        "qw8": _w8(wq.T),
        "kw8": _w8(wk.T),
        "vw8": _w8(wv.T),
        "pw8": _w8(proj_w.T),
        "wn": np.ascontiguousarray(wn.astype(ml_dtypes.bfloat16)),
        "biases": np.ascontiguousarray(
            np.stack([qkv_b[0:C] + wq @ norm_b, qkv_b[C:2 * C] + wk @ norm_b,
                      vb_full,
                      proj_b + proj_w @ vb_full])),
        "nwb": np.ascontiguousarray(np.stack([norm_w, norm_b])),
        "mask8": np.ascontiguousarray(
            (np.arange(128)[:, None] // GSIZE == np.arange(8)[None, :]).astype(np.float32)),
        "mask8t": np.ascontiguousarray(
            (np.arange(128)[None, :] // GSIZE == np.arange(8)[:, None]).astype(np.float32)),
    }

    in_maps = []
    for core in range(8):
        b, ih = core // 2, core % 2
        xb = x[b].reshape(C, S)
        if ih == 1:
            xb = np.concatenate([xb[:, IHALF:], xb[:, :IHALF]], axis=1)
        m = dict(shared)
        m["x"] = np.ascontiguousarray(xb)
        m["x8"] = np.ascontiguousarray(xb.astype(ml_dtypes.float8_e4m3))
        in_maps.append(m)
    return in_maps


def kernel(**inputs):
    nc = _get_nc(1)
    in_maps = _make_in_maps(inputs)
    res = run_bass_kernel_spmd(nc, in_maps, core_ids=list(range(8)))
    y = np.empty((B, C, S), dtype=np.float32)
    for core in range(8):
        b, ih = core // 2, core % 2
        y[b][:, ih * IHALF:(ih + 1) * IHALF] = res.results[core]["out"]
    return y.reshape(B, C, HH, WW)


# revision 18
# speedup vs baseline: 1.2351x; 1.2351x over previous
"""Trainium2 Bass kernel for nn_AttentionBlock (GroupNorm + 1x1-conv QKV +
softmax attention + 1x1-conv proj + residual), B=4 C=512 H=W=64 HEADS=8.

Sharding: 8 cores = (batch b in 0..4) x (query-half ih in 0..2).  Each core
computes groupnorm + K/V for its whole batch (duplicated across the 2 cores
sharing a batch -- cheap) and attention + proj for its 2048 queries.  Cores
are fully independent SPMD (no collectives); the host splits and concats.
For ih=1 cores the host *rolls* the spatial columns of x by 2048 so that the
query half is always columns [0:2048).

Key optimizations over the bf16 baseline:
- The softmax exp (67M elems/core) is split across ScalarE (LUT exp,
  fp8e4 out) and DVE (Schraudolph exp: one tensor_scalar f32->int8 with
  truncation, bit-cast as fp8e4m3).  P stays fp8 everywhere.
- O = P @ V_ext runs in fp8 DoubleRow mode (contraction 256 = two key
  tiles per instruction) -- 2x fewer PE columns streamed than bf16.
- QKV and proj matmuls also fp8 DoubleRow (weights pre-scaled x256 on the
  host to dodge fp8 subnormals; the 1/256 is folded into the PSUM evac).
- Per-(head-pair, query-block) attention: scores bf16 with 2-head PE row
  packing (auto tile_position row tiling); jt-pair software pipeline feeds
  DoubleRow O one step behind.
- Softmax denominator: O is evacuated UNNORMALIZED (x 1/16, fp8); the
  colsum rows are gathered into an [8, 2048] SBUF tile, inverted once per
  head-pair with a single DVE reciprocal_approx_fast, broadcast to 128
  partitions via a DRAM-roundtrip DMA, and applied with one batched
  [128, 2048] DVE multiply per head-pair.  This removes all Ln/Exp
  activations from the attention loop, which kills the ACT table-set
  thrashing (natural_log vs exp_and_others reload = 1.3us each) that the
  old per-block exp(-ln(cs)) reciprocal caused.
- sc PSUM tiles triple-buffered (the exp(jt-3) -> scores MM(jt) -> exp(jt)
  recycle loop must be longer than one exp); block n's o_ps evacuation is
  deferred into the first jts of block n+1.
- GroupNorm rstd = exp(-0.5*ln(var+eps)) is batched into one Ln + one Exp
  over all 4 channel tiles (2 table loads total, in the prologue).
- GPSIMD (no PSUM access) takes SBUF->SBUF work: groupnorm normalize,
  proj residual add.  QKV PSUM evacuations are split ACT/DVE.
- x is DMA'd once and stays resident in SBUF (stats + normalize + residual).
"""

from contextlib import ExitStack

import numpy as np
import ml_dtypes

import concourse.bass as bass
import concourse.tile as tile
import concourse.mybir as mybir
from concourse import bacc
from concourse.bass_utils import run_bass_kernel_spmd

F32 = mybir.dt.float32
BF16 = mybir.dt.bfloat16
F8 = mybir.dt.float8e4
I8 = mybir.dt.int8
EXP = mybir.ActivationFunctionType.Exp
IDENT = mybir.ActivationFunctionType.Identity
LN = mybir.ActivationFunctionType.Ln
MULT = mybir.AluOpType.mult
ADD = mybir.AluOpType.add
DR = mybir.MatmulPerfMode.DoubleRow

B, C, HH, WW = 4, 512, 64, 64
S = HH * WW              # 4096
HEADS = 8
HD = C // HEADS          # 64
GROUPS = 32
GSIZE = C // GROUPS      # 16 channels per group
EPS = 1e-5
SCALE = 1.0 / 8.0        # 1/sqrt(head_dim)
IHALF = S // 2           # 2048 queries per core
CT = C // 128            # 4 channel tiles
ST = S // 128            # 32 spatial (key) tiles
WSCALE = 256.0           # host-side fp8 weight pre-scale
OSCALE = 1.0 / 16.0      # unnormalized-O evac scale (fp8 headroom)
IWS = 512                # query block
NIW = IHALF // IWS       # 4 query blocks

# Schraudolph exp2 constants for fp8e4m3 bit pattern (HW f32->int8 convert
# rounds to nearest): bits8 = round(s * A8 + B8);  A8 = 2^3*log2(e)*SCALE
A8 = 8.0 * 1.4426950408889634 * SCALE
B8 = 56.0 - 0.42

# per-jt exp engine: 'A' = ScalarE LUT exp, 'D' = DVE Schraudolph
def _mk_pat(counts):
    tot = sum(counts.values())
    acc = {k: 0.0 for k in counts}
    out = []
    for _ in range(tot):
        for k in counts:
            acc[k] += counts[k] / tot
        k = max(acc, key=lambda q: acc[q])
        acc[k] -= 1.0
        out.append(k)
    return "".join(out)


EXP_PAT = _mk_pat({"A": 18, "D": 14})
assert len(EXP_PAT) == ST


import os
DBG = bool(os.environ.get("K_DBG"))


def build_kernel(reps: int = 1):
    nc = bacc.Bacc("TRN2", target_bir_lowering=False, debug=False)

    x_d = nc.dram_tensor("x", [C, S], F32, kind="ExternalInput").ap()
    x8_d = nc.dram_tensor("x8", [C, S], F8, kind="ExternalInput").ap()
    wn_d = nc.dram_tensor("wn", [4, GROUPS, C], BF16, kind="ExternalInput").ap()
    beta_dram = nc.dram_tensor("beta_scratch", [GROUPS, 1], BF16, kind="Internal").ap()
    qw_d = nc.dram_tensor("qw8", [C, C], F8, kind="ExternalInput").ap()   # (qkv_w[0:512].T * 256)
    kw_d = nc.dram_tensor("kw8", [C, C], F8, kind="ExternalInput").ap()
    vw_d = nc.dram_tensor("vw8", [C, C], F8, kind="ExternalInput").ap()
    pw_d = nc.dram_tensor("pw8", [C, C], F8, kind="ExternalInput").ap()   # proj_w.T * 256
    bias_d = nc.dram_tensor("biases", [4, C], F32, kind="ExternalInput").ap()  # qb,kb,vb,pb
    nwb_d = nc.dram_tensor("nwb", [2, C], F32, kind="ExternalInput").ap()      # norm_w, norm_b
    m8_d = nc.dram_tensor("mask8", [128, 8], F32, kind="ExternalInput").ap()
    m8t_d = nc.dram_tensor("mask8t", [8, 128], F32, kind="ExternalInput").ap()
    cs_dram = nc.dram_tensor("cs_scratch", [8, IHALF], F32, kind="Internal").ap()
    out_d = nc.dram_tensor("out", [C, IHALF], F32, kind="ExternalOutput").ap()
    dbg = {}
    if DBG:
        dbg["h"] = nc.dram_tensor("dbg_h", [128, CT, S], F8, kind="ExternalOutput").ap()
        dbg["k"] = nc.dram_tensor("dbg_k", [128, CT, S], BF16, kind="ExternalOutput").ap()
        dbg["q"] = nc.dram_tensor("dbg_q", [128, CT, IHALF], BF16, kind="ExternalOutput").ap()
        dbg["vt"] = nc.dram_tensor("dbg_vt", [128, ST, HEADS, 66], F8, kind="ExternalOutput").ap()
        dbg["ex"] = nc.dram_tensor("dbg_ex", [128, 2, 2, 2, IWS], F8, kind="ExternalOutput").ap()
        dbg["on"] = nc.dram_tensor("dbg_on", [128, CT, IHALF], F8, kind="ExternalOutput").ap()
        dbg["cs"] = nc.dram_tensor("dbg_cs", [4, 2, IHALF], F32, kind="ExternalOutput").ap()

    with tile.TileContext(nc) as tc:
        with ExitStack() as ctx:
            const = ctx.enter_context(tc.tile_pool(name="const", bufs=1))

            wts = {}
            for nm, ap in [("qw", qw_d), ("kw", kw_d), ("vw", vw_d), ("pw", pw_d)]:
                t8 = const.tile([128, CT, C], F8, name=f"{nm}_f8")
                nc.sync.dma_start(out=t8, in_=ap.rearrange("(kt p) c -> p kt c", p=128))
                wts[nm] = t8

            bias_cols = const.tile([128, 4, CT], F32)
            nc.sync.dma_start(out=bias_cols,
                              in_=bias_d.rearrange("w (kt p) -> p w kt", p=128))
            nwb_t = const.tile([128, 2, CT], F32)
            nc.sync.dma_start(out=nwb_t, in_=nwb_d.rearrange("w (kt p) -> p w kt", p=128))

            wn_t = const.tile([GROUPS, 4, CT, 128], BF16)
            nc.sync.dma_start(out=wn_t,
                              in_=wn_d.rearrange("w g (mt p) -> g w mt p", p=128))

            m8_t = const.tile([128, 8], F32)
            nc.sync.dma_start(out=m8_t, in_=m8_d)
            m8t_t = const.tile([8, 128], F32)
            nc.sync.dma_start(out=m8t_t, in_=m8t_d)

            eps_t = const.tile([8, 1], F32)
            nc.vector.memset(eps_t, EPS)

            consts = (wts, wn_t, bias_cols, nwb_t, m8_t, m8t_t, eps_t)
            if reps == 1:
                _one_pass(nc, tc, x_d, x8_d, out_d, cs_dram, beta_dram, consts, dbg)
            else:
                with tc.For_i(0, reps, 1):
                    _one_pass(nc, tc, x_d, x8_d, out_d, cs_dram, beta_dram, consts, dbg)
    nc.compile()
    return nc


def _one_pass(nc, tc, x_d, x8_d, out_d, cs_dram, beta_dram, consts, dbg={}):
    (wts, wn_t, bias_cols, nwb_t, m8_t, m8t_t, eps_t) = consts
    with ExitStack() as ps:
        big = ps.enter_context(tc.tile_pool(name="big", bufs=1))
        x8_all = big.tile([128, CT, S], F8)             # 16 KB (host fp8 cast)
        wsc = {nm: big.tile([128, CT, C], F8, name=f"{nm}_sc")
               for nm in ("qw", "kw", "vw")}            # affine-scaled weights
        bias_eff = big.tile([128, 4, CT], F32)          # q/k/v/proj biases after absorb
        k_all = big.tile([128, CT, S], BF16)            # 32 KB
        q_all = big.tile([128, CT, IHALF], BF16)        # 16 KB
        vt_ext = big.tile([128, ST, HEADS, 66], F8)     # ~16.5 KB
        o_raw8 = big.tile([128, CT, IHALF], F8)         # 8 KB  (unnormalized o/16)
        o_norm = big.tile([128, CT, IHALF], F8)         # 8 KB
        ex_all = big.tile([128, 2, 2, 2, IWS], F8)      # (head, jtpbuf, t, cols) 2 KB

        svec = ps.enter_context(tc.tile_pool(name="svec", bufs=1))
        aff_s = svec.tile([128, CT], F32)
        aff_t = svec.tile([128, CT], F32)
        # colsums/16 for (hp, p) at partition 32*hp, free dim p (32-aligned
        # partition bases are required for engine operands); the reciprocal
        # runs in place per head-pair.
        cs_ab = svec.tile([128, 2, IHALF], F32)

        # =============== P1: x8 load + groupnorm statistics ===============
        # Stats run on the host-cast fp8 x (stat noise ~1e-3 rel, well inside
        # budget) so the 8MB f32 x never enters SBUF.  The residual is
        # instead pre-copied DRAM->DRAM into out and the proj result is
        # DMA-accumulated onto it at the end.
        for t in range(CT):
            eng = nc.sync if t % 2 == 0 else nc.scalar
            eng.dma_start(out=x8_all[:, t, :], in_=x8_d[t * 128:(t + 1) * 128, :])

        with tc.tile_pool(name="stat", bufs=2) as stat, \
             tc.tile_pool(name="gn_ps", bufs=1, space="PSUM") as gn_ps:
            dummy_ln = stat.tile([8, 1], F32, tag="dln", bufs=1)
            nc.scalar.activation(dummy_ln, eps_t, LN)
            gstats = gn_ps.tile([8, CT, 2], F32)
            warm_ps = gn_ps.tile([6, 512], F32)
            warm_src = stat.tile([128, 6], F8, tag="wsrc", bufs=1)
            for t in range(CT):
                bnst = stat.tile([128, 8, 6], F32, tag="bnst")
                for sg in range(8):
                    nc.vector.bn_stats(out=bnst[:, sg, :],
                                       in_=x8_all[:, t, sg * 512:(sg + 1) * 512])
                mv = stat.tile([128, 2], F32, tag="mv")
                nc.vector.bn_aggr(out=mv, in_=bnst)
                st_t = stat.tile([128, 2], F32, tag="st")   # (mean, 2nd moment)
                nc.vector.tensor_copy(st_t[:, 0:1], mv[:, 0:1])
                sq = stat.tile([128, 1], F32, tag="sq")
                nc.vector.tensor_mul(sq, mv[:, 0:1], mv[:, 0:1])
                nc.vector.tensor_add(st_t[:, 1:2], mv[:, 1:2], sq)
                nc.tensor.matmul(gstats[:, t, :], m8_t, st_t, start=True, stop=True)
                if t == 0:
                    nc.vector.tensor_copy(warm_src, bnst[:, 0, :])
            # PE keep-warm dummies after the gstats chain (bridge to qkv)
            for _ in range(2):
                nc.tensor.matmul(warm_ps, warm_src, x8_all[:, 0, 0:512],
                                 start=True, stop=True)

            gs = stat.tile([8, CT, 2], F32, tag="gs")
            nc.vector.tensor_copy(gs, gstats)
            grp = stat.tile([8, CT, 2], F32, tag="grp")   # (mu_g, rstd_g)
            var_all = stat.tile([8, CT], F32, tag="var")
            for t in range(CT):
                nc.scalar.mul(grp[:, t, 0:1], gs[:, t, 0:1], 1.0 / GSIZE)
                e_t = stat.tile([8, 1], F32, tag="e")
                nc.scalar.mul(e_t, gs[:, t, 1:2], 1.0 / GSIZE)
                musq = stat.tile([8, 1], F32, tag="musq")
                nc.vector.tensor_mul(musq, grp[:, t, 0:1], grp[:, t, 0:1])
                nc.vector.tensor_sub(var_all[:, t:t + 1], e_t, musq)
            # rstd = exp(-0.5*ln(var+eps)), batched over all CT tiles:
            # one Ln + one Exp = exactly two ACT table loads, in the prologue.
            lnv = stat.tile([8, CT], F32, tag="lnv")
            nc.scalar.activation(lnv, var_all, LN, bias=eps_t)
            nc.scalar.activation(grp[:, :, 1], lnv, EXP, scale=-0.5)

            bc_ps = gn_ps.tile([128, CT, 2], F32)
            for t in range(CT):
                nc.tensor.matmul(bc_ps[:, t, :], m8t_t, grp[:, t, :], start=True, stop=True)
            bc = stat.tile([128, CT, 2], F32, tag="bc")
            nc.vector.tensor_copy(bc, bc_ps)
            nc.vector.tensor_mul(aff_s, nwb_t[:, 0, :], bc[:, :, 1])
            # beta_g = mu_g * rstd_g -> [32, 1] via DRAM roundtrip (engines
            # cannot re-map partitions; DMA can)
            beta8 = stat.tile([8, CT], BF16, tag="beta8")
            nc.vector.tensor_mul(beta8, grp[:, :, 0], grp[:, :, 1])
            wb = nc.sync.dma_start(out=beta_dram.rearrange("(kt i) o -> i (kt o)", i=8),
                                   in_=beta8)
            tc.chain_iter_dep("betadram", wb.ins)
            beta32 = stat.tile([GROUPS, 1], BF16, tag="beta32")
            rb_ = nc.sync.dma_start(out=beta32, in_=beta_dram)
            tc.chain_iter_dep("betadram", rb_.ins)

        # =============== P2: absorb groupnorm affine into qkv weights ======
        # w'[o,c] = w[o,c] * s[c] (per-partition scale of the stationary
        # side); bias' = b + W@norm_b (host) - Wn @ beta (12 tiny f32 MMs).
        for i, nm in enumerate(("qw", "kw", "vw")):
            for kt in range(CT):
                if (i + kt) % 2 == 0:
                    nc.scalar.activation(wsc[nm][:, kt, :], wts[nm][:, kt, :],
                                         IDENT, scale=aff_s[:, kt:kt + 1])
                else:
                    nc.vector.tensor_scalar(
                        out=wsc[nm][:, kt, :], in0=wts[nm][:, kt, :],
                        scalar1=aff_s[:, kt:kt + 1], scalar2=None, op0=MULT)
        with tc.tile_pool(name="bias_ps", bufs=1, space="PSUM") as bias_ps:
            bps = bias_ps.tile([128, 4, CT], F32)
            for w in range(4):
                for mt in range(CT):
                    nc.tensor.matmul(bps[:, w, mt:mt + 1], wn_t[:, w, mt, :],
                                     beta32, start=True, stop=True)
            nc.vector.tensor_sub(bias_eff, bias_cols, bps)

        nc.vector.memset(vt_ext[:, :, :, 64:66], 0.0)
        nc.vector.memset(vt_ext[:, :, :, 64:65], 1.0)

        # =============== P3: qkv projections (fp8 DoubleRow) ===============
        with tc.tile_pool(name="qkv_ps", bufs=3, space="PSUM") as qkv_ps:
            for dst, w8, bidx, ncols in ((k_all, wsc["kw"], 1, S),
                                         (q_all, wsc["qw"], 0, IHALF)):
                for mt in range(CT):
                    for ic2 in range(ncols // 1024):
                        acc = qkv_ps.tile([128, 1024], F32, tag="qkps")
                        for j in range(2):
                            icol = 2 * ic2 + j
                            for kp in range(2):
                                nc.tensor.matmul(
                                    acc[:, j * 512:(j + 1) * 512],
                                    w8[:, 2 * kp:2 * kp + 2, mt * 128:(mt + 1) * 128],
                                    x8_all[:, 2 * kp:2 * kp + 2, icol * 512:(icol + 1) * 512],
                                    start=(kp == 0), stop=(kp == 1), perf_mode=DR)
                        # one 1024-wide evac per pair, split ACT/DVE
                        if (mt + ic2) % 2 == 0:
                            nc.scalar.activation(
                                dst[:, mt, ic2 * 1024:(ic2 + 1) * 1024], acc, IDENT,
                                scale=1.0 / WSCALE, bias=bias_eff[:, bidx, mt:mt + 1])
                        else:
                            nc.vector.tensor_scalar(
                                out=dst[:, mt, ic2 * 1024:(ic2 + 1) * 1024], in0=acc,
                                scalar1=1.0 / WSCALE, scalar2=bias_eff[:, bidx, mt:mt + 1],
                                op0=MULT, op1=ADD)
            # V^T: [S, 512c] scattered into vt_ext (64 cols per head + ones
            # col).  The V bias is folded into the proj bias on the host
            # (pb' = pb + proj_w @ vb), so the evac is a pure scale and can
            # split across ACT and DVE.
            for sp in range(ST // 2):
                acc = qkv_ps.tile([128, 1024], F32, tag="qkps")
                for j in range(2):
                    st = 2 * sp + j
                    for kp in range(2):
                        nc.tensor.matmul(
                            acc[:, j * 512:(j + 1) * 512],
                            x8_all[:, 2 * kp:2 * kp + 2, st * 128:(st + 1) * 128],
                            wsc["vw"][:, 2 * kp:2 * kp + 2, :],
                            start=(kp == 0), stop=(kp == 1), perf_mode=DR)
                if sp % 2 == 0:
                    nc.scalar.activation(
                        vt_ext[:, 2 * sp:2 * sp + 2, :, 0:64],
                        acc.rearrange("p (s h d) -> p s h d", s=2, d=HD), IDENT,
                        scale=1.0 / WSCALE)
                else:
                    nc.vector.tensor_scalar(
                        out=vt_ext[:, 2 * sp:2 * sp + 2, :, 0:64],
                        in0=acc.rearrange("p (s h d) -> p s h d", s=2, d=HD),
                        scalar1=1.0 / WSCALE, scalar2=None, op0=MULT)

        if dbg:
            nc.sync.dma_start(out=dbg["h"], in_=x8_all)
            nc.sync.dma_start(out=dbg["k"], in_=k_all)
            nc.sync.dma_start(out=dbg["q"], in_=q_all)
            nc.sync.dma_start(out=dbg["vt"], in_=vt_ext)

        # residual precopy: out <- x (runs during attention; bus is idle)
        xc = nc.scalar.dma_start(out=out_d, in_=x_d[:, 0:IHALF])
        for mt in range(CT):
            tc.chain_iter_dep(f"xres{mt}", xc.ins)

        # =============== P4: attention ===============
        # Per (head-pair hp, query block iw of 512): scores bf16 with 2-head
        # row packing -> PSUM [128, 2, 512]; exp on ScalarE/DVE per EXP_PAT
        # -> ex fp8; O accumulates per head in fp8 DoubleRow over jt pairs,
        # pipelined one jt-pair behind the scores/exp.  o_ps double-buffered
        # across blocks; evacuation of block n is issued early in block n+1.
        with tc.tile_pool(name="att_ps", bufs=1, space="PSUM") as att_ps, \
             tc.tile_pool(name="att_sb", bufs=1) as att_sb:
            o_ps = [att_ps.tile([65, IWS], F32, tag=f"o{p}", name=f"o_ps{p}")
                    for p in range(2)]
            rb_t = att_sb.tile([128, IHALF], F32, name="rb")

            blocks = [(hp, iw) for hp in range(CT) for iw in range(NIW)]
            deferred = {"A": [], "D": []}   # evac closures from previous block
            chain = {}                      # jt -> closure (recip / dma / mult)

            def mk_evac(hp, iw, o_ps):
                # evacuate unnormalized o (x 1/16) + gather colsum rows
                def evac_a():
                    nc.scalar.activation(
                        o_raw8[0:64, hp, iw * IWS:(iw + 1) * IWS],
                        o_ps[0][0:64, :], IDENT, scale=OSCALE)
                    nc.scalar.activation(
                        cs_ab[32 * hp:32 * hp + 1, 0, iw * IWS:(iw + 1) * IWS],
                        o_ps[0][64:65, :], IDENT, scale=OSCALE)
                def evac_d():
                    nc.vector.tensor_scalar(
                        out=o_raw8[64:128, hp, iw * IWS:(iw + 1) * IWS],
                        in0=o_ps[1][0:64, :], scalar1=OSCALE, scalar2=None, op0=MULT)
                    nc.vector.tensor_scalar(
                        out=cs_ab[32 * hp:32 * hp + 1, 1, iw * IWS:(iw + 1) * IWS],
                        in0=o_ps[1][64:65, :], scalar1=OSCALE, scalar2=None, op0=MULT)
                return evac_a, evac_d

            def mk_chain(hp, rb):
                # 16/cs for head-pair hp: DRAM-roundtrip broadcast of the raw
                # colsums to 128 partitions, then one in-place DVE
                # reciprocal_approx_fast on the broadcast tile (base partition
                # 0 -- the custom-DVE op mis-addresses nonzero base
                # partitions; engine cost is free-size only, so recip on
                # [128, 2048] costs the same as on [1, 2048]), then one
                # batched normalize multiply.
                def c_dma():
                    w = nc.sync.dma_start(out=cs_dram[2 * hp:2 * hp + 2, :],
                                          in_=cs_ab[32 * hp:32 * hp + 1, :, :])
                    tc.chain_iter_dep("csdram", w.ins)
                    b0 = nc.sync.dma_start(
                        out=rb[0:64, :],
                        in_=cs_dram[2 * hp:2 * hp + 1, :].to_broadcast([64, IHALF]))
                    tc.chain_iter_dep("csdram", b0.ins)
                    b1 = nc.sync.dma_start(
                        out=rb[64:128, :],
                        in_=cs_dram[2 * hp + 1:2 * hp + 2, :].to_broadcast([64, IHALF]))
                    tc.chain_iter_dep("csdram", b1.ins)
                def c_recip():
                    nc.vector.reciprocal_approx_fast(out=rb, in_=rb)
                def c_mult():
                    nc.vector.tensor_mul(o_norm[:, hp, :], o_raw8[:, hp, :], rb)
                return [c_dma, c_recip, c_mult]

            for bi, (hp, iw) in enumerate(blocks):
                def emit_o(jtp, p):
                    nc.tensor.matmul(
                        o_ps[p],
                        vt_ext[:, 2 * jtp:2 * jtp + 2, 2 * hp + p, 0:65],
                        ex_all[:, p, jtp % 2, :, :],
                        start=(jtp == 0), stop=(jtp == ST // 2 - 1),
                        perf_mode=DR, skip_group_check=True)

                for jtp in range(ST // 2):
                    for t in range(2):
                        jt = 2 * jtp + t
                        sc = att_ps.tile([128, 2, IWS], F32, tag=f"sc{jt % 3}",
                                         name=f"sc{jt % 3}")
                        for p in range(2):
                            nc.tensor.matmul(
                                sc[:, p, :],
                                k_all[64 * p:64 * p + 64, hp, jt * 128:(jt + 1) * 128],
                                q_all[64 * p:64 * p + 64, hp, iw * IWS:(iw + 1) * IWS],
                                start=True, stop=True)
                        ex_ap = ex_all[:, :, jtp % 2, t, :]
                        if EXP_PAT[jt] == "A":
                            nc.scalar.activation(ex_ap, sc, EXP, scale=SCALE)
                        else:
                            nc.vector.tensor_scalar(
                                out=ex_ap.bitcast(I8), in0=sc,
                                scalar1=A8, scalar2=B8, op0=MULT, op1=ADD)
                        # flush previous block's evacuations / recip chain
                        if jt == 1 and deferred["A"]:
                            for f in deferred["A"]:
                                f()
                            deferred["A"] = []
                        elif jt == 2 and deferred["D"]:
                            for f in deferred["D"]:
                                f()
                            deferred["D"] = []
                        elif jt in chain:
                            chain.pop(jt)()
                    if jtp > 0:
                        for p in range(2):
                            emit_o(jtp - 1, p)
                    if dbg and hp == 0 and iw == 0 and jtp == 0:
                        nc.sync.dma_start(out=dbg["ex"], in_=ex_all)
                for p in range(2):
                    emit_o(ST // 2 - 1, p)

                evac_a, evac_d = mk_evac(hp, iw, o_ps)
                if bi == len(blocks) - 1:
                    evac_a(); evac_d()
                    for f in mk_chain(hp, rb_t):
                        f()
                else:
                    deferred["A"].append(evac_a)
                    deferred["D"].append(evac_d)
                    if iw == NIW - 1:
                        # schedule hp's normalize chain into upcoming blocks
                        fns = mk_chain(hp, rb_t)
                        chain[6] = fns[0]    # dma    (block bi+1, jt 6)
                        chain[18] = fns[1]   # recip  (block bi+1, jt 18)
                        chain[20] = fns[2]   # mult   (block bi+1, jt 20)


        if dbg:
            nc.sync.dma_start(out=dbg["on"], in_=o_norm)
            for hp in range(CT):
                nc.sync.dma_start(out=dbg["cs"][hp], in_=cs_ab[32 * hp:32 * hp + 1, :, :])

        # =============== P5: proj + residual ===============
        # out already holds x (DRAM precopy); evac applies scale+bias, then a
        # gpsimd SWDGE DMA accumulates the proj result onto out (one per mt).
        with tc.tile_pool(name="pj_ps", bufs=3, space="PSUM") as pj_ps, \
             tc.tile_pool(name="pj_sb", bufs=2) as pj_sb:
            for mt in range(CT):
                o_mt = pj_sb.tile([128, IHALF], F32, tag="ot")
                for icol in range(IHALF // 512):
                    acc = pj_ps.tile([128, 512], F32, tag="pj")
                    for kp in range(2):
                        nc.tensor.matmul(
                            acc, wts["pw"][:, 2 * kp:2 * kp + 2, mt * 128:(mt + 1) * 128],
                            o_norm[:, 2 * kp:2 * kp + 2, icol * 512:(icol + 1) * 512],
                            start=(kp == 0), stop=(kp == 1), perf_mode=DR)
                    if icol % 2 == 0:
                        nc.scalar.activation(
                            o_mt[:, icol * 512:(icol + 1) * 512], acc, IDENT,
                            scale=1.0 / WSCALE, bias=bias_eff[:, 3, mt:mt + 1])
                    else:
                        nc.vector.tensor_scalar(
                            out=o_mt[:, icol * 512:(icol + 1) * 512], in0=acc,
                            scalar1=1.0 / WSCALE, scalar2=bias_eff[:, 3, mt:mt + 1],
                            op0=MULT, op1=ADD)
                ac = nc.gpsimd.dma_start(
                    out=out_d[mt * 128:(mt + 1) * 128, :], in_=o_mt,
                    accum_op=ADD)
                tc.chain_iter_dep(f"xres{mt}", ac.ins)


# ---------------------------------------------------------------------------
# host side
# ---------------------------------------------------------------------------

_CACHE = {}


def _get_nc(reps=1):
    if reps not in _CACHE:
        _CACHE[reps] = build_kernel(reps)
    return _CACHE[reps]


def _w8(a):
    return np.ascontiguousarray((a * WSCALE).astype(ml_dtypes.float8_e4m3))


def _make_in_maps(inputs):
    x = np.ascontiguousarray(np.asarray(inputs["x"], dtype=np.float32))
    qkv_w = np.asarray(inputs["qkv_w"], dtype=np.float32)
    qkv_b = np.asarray(inputs["qkv_b"], dtype=np.float32)
    proj_w = np.asarray(inputs["proj_w"], dtype=np.float32)
    proj_b = np.asarray(inputs["proj_b"], dtype=np.float32)
    norm_w = np.asarray(inputs["norm_w"], dtype=np.float32)
    norm_b = np.asarray(inputs["norm_b"], dtype=np.float32)

    wq, wk, wv = qkv_w[0:C], qkv_w[C:2 * C], qkv_w[2 * C:3 * C]
    # group-collapsed weights (for the on-device bias correction -Wn@beta)
    wn3 = [(w * norm_w[None, :]).reshape(C, GROUPS, GSIZE).sum(axis=2)
           for w in (wq, wk, wv)]                      # each [512(out), 32(g)]
    wn = np.stack([wn3[0].T, wn3[1].T, wn3[2].T,
                   (proj_w @ wn3[2]).T])               # [4, 32, 512]
    vb_full = qkv_b[2 * C:3 * C] + wv @ norm_b   # v-bias base (beta part on dev)
    shared = {
        "qw8": _w8(wq.T),
        "kw8": _w8(wk.T),
        "vw8": _w8(wv.T),
        "pw8": _w8(proj_w.T),
        "wn": np.ascontiguousarray(wn.astype(ml_dtypes.bfloat16)),
        "biases": np.ascontiguousarray(
            np.stack([qkv_b[0:C] + wq @ norm_b, qkv_b[C:2 * C] + wk @ norm_b,
                      vb_full,
                      proj_b + proj_w @ vb_full])),
        "nwb": np.ascontiguousarray(np.stack([norm_w, norm_b])),
        "mask8": np.ascontiguousarray(
            (np.arange(128)[:, None] // GSIZE == np.arange(8)[None, :]).astype(np.float32)),
        "mask8t": np.ascontiguousarray(
            (np.arange(128)[None, :] // GSIZE == np.arange(8)[:, None]).astype(np.float32)),
    }

    in_maps = []
    for core in range(8):
        b, ih = core // 2, core % 2
        xb = x[b].reshape(C, S)
        if ih == 1:
            xb = np.concatenate([xb[:, IHALF:], xb[:, :IHALF]], axis=1)
        m = dict(shared)
        m["x"] = np.ascontiguousarray(xb)
        m["x8"] = np.ascontiguousarray(xb.astype(ml_dtypes.float8_e4m3))
        in_maps.append(m)
    return in_maps


def kernel(**inputs):
    nc = _get_nc(1)
    in_maps = _make_in_maps(inputs)
    res = run_bass_kernel_spmd(nc, in_maps, core_ids=list(range(8)))
    y = np.empty((B, C, S), dtype=np.float32)
    for core in range(8):
        b, ih = core // 2, core % 2
        y[b][:, ih * IHALF:(ih + 1) * IHALF] = res.results[core]["out"]
    return y.reshape(B, C, HH, WW)


# revision 19
# speedup vs baseline: 1.7006x; 1.3769x over previous
"""Trainium2 Bass kernel for nn_AttentionBlock (GroupNorm + 1x1-conv QKV +
softmax attention + 1x1-conv proj + residual), B=4 C=512 H=W=64 HEADS=8.

Sharding: 8 cores = (batch b in 0..4) x (query-half ih in 0..2).  Each core
computes groupnorm + K/V for its whole batch (duplicated across the 2 cores
sharing a batch -- cheap) and attention + proj for its 2048 queries.  Cores
are fully independent SPMD (no collectives); the host splits and concats.
For ih=1 cores the host *rolls* the spatial columns of x by 2048 so that the
query half is always columns [0:2048).

Key optimizations over the bf16 baseline:
- The softmax exp (67M elems/core) is split across ScalarE (LUT exp,
  fp8e4 out) and DVE (Schraudolph exp: one tensor_scalar f32->int8 with
  truncation, bit-cast as fp8e4m3).  P stays fp8 everywhere.
- O = P @ V_ext runs in fp8 DoubleRow mode (contraction 256 = two key
  tiles per instruction) -- 2x fewer PE columns streamed than bf16.
- QKV and proj matmuls also fp8 DoubleRow (weights pre-scaled x256 on the
  host to dodge fp8 subnormals; the 1/256 is folded into the PSUM evac).
- Per-(head-pair, query-block) attention: scores bf16 with 2-head PE row
  packing (auto tile_position row tiling); jt-pair software pipeline feeds
  DoubleRow O one step behind.
- Softmax denominator: O is evacuated UNNORMALIZED (x 1/16, fp8); the
  colsum rows are gathered into an [8, 2048] SBUF tile, inverted once per
  head-pair with a single DVE reciprocal_approx_fast, broadcast to 128
  partitions via a DRAM-roundtrip DMA, and applied with one batched
  [128, 2048] DVE multiply per head-pair.  This removes all Ln/Exp
  activations from the attention loop, which kills the ACT table-set
  thrashing (natural_log vs exp_and_others reload = 1.3us each) that the
  old per-block exp(-ln(cs)) reciprocal caused.
- sc PSUM tiles triple-buffered (the exp(jt-3) -> scores MM(jt) -> exp(jt)
  recycle loop must be longer than one exp); block n's o_ps evacuation is
  deferred into the first jts of block n+1.
- GroupNorm rstd = exp(-0.5*ln(var+eps)) is batched into one Ln + one Exp
  over all 4 channel tiles (2 table loads total, in the prologue).
- GPSIMD (no PSUM access) takes SBUF->SBUF work: groupnorm normalize,
  proj residual add.  QKV PSUM evacuations are split ACT/DVE.
- x is DMA'd once and stays resident in SBUF (stats + normalize + residual).
"""

from contextlib import ExitStack

import numpy as np
import ml_dtypes

import concourse.bass as bass
import concourse.tile as tile
import concourse.mybir as mybir
from concourse import bacc
from concourse.bass_utils import run_bass_kernel_spmd

F32 = mybir.dt.float32
BF16 = mybir.dt.bfloat16
F8 = mybir.dt.float8e4
I8 = mybir.dt.int8
EXP = mybir.ActivationFunctionType.Exp
IDENT = mybir.ActivationFunctionType.Identity
LN = mybir.ActivationFunctionType.Ln
MULT = mybir.AluOpType.mult
ADD = mybir.AluOpType.add
DR = mybir.MatmulPerfMode.DoubleRow

B, C, HH, WW = 4, 512, 64, 64
S = HH * WW              # 4096
HEADS = 8
HD = C // HEADS          # 64
GROUPS = 32
GSIZE = C // GROUPS      # 16 channels per group
EPS = 1e-5
SCALE = 1.0 / 8.0        # 1/sqrt(head_dim)
IHALF = S // 2           # 2048 queries per core
CT = C // 128            # 4 channel tiles
ST = S // 128            # 32 spatial (key) tiles
WSCALE = 256.0           # host-side fp8 weight pre-scale
OSCALE = 1.0 / 16.0      # unnormalized-O evac scale (fp8 headroom)
IWS = 512                # query block
NIW = IHALF // IWS       # 4 query blocks

# Schraudolph exp2 constants for fp8e4m3 bit pattern (HW f32->int8 convert
# rounds to nearest): bits8 = round(s * A8 + B8);  A8 = 2^3*log2(e)*SCALE
A8 = 8.0 * 1.4426950408889634 * SCALE
B8 = 56.0 - 0.42

# per-jt exp engine: 'A' = ScalarE LUT exp, 'D' = DVE Schraudolph
def _mk_pat(counts):
    tot = sum(counts.values())
    acc = {k: 0.0 for k in counts}
    out = []
    for _ in range(tot):
        for k in counts:
            acc[k] += counts[k] / tot
        k = max(acc, key=lambda q: acc[q])
        acc[k] -= 1.0
        out.append(k)
    return "".join(out)


EXP_PAT = _mk_pat({"A": 18, "D": 14})
assert len(EXP_PAT) == ST


import os
DBG = bool(os.environ.get("K_DBG"))


def build_kernel(reps: int = 1):
    nc = bacc.Bacc("TRN2", target_bir_lowering=False, debug=False)

    x_d = nc.dram_tensor("x", [C, S], F32, kind="ExternalInput").ap()
    x8_d = nc.dram_tensor("x8", [C, S], F8, kind="ExternalInput").ap()
    wn_d = nc.dram_tensor("wn", [4, GROUPS, C], BF16, kind="ExternalInput").ap()
    beta_dram = nc.dram_tensor("beta_scratch", [GROUPS, 1], BF16, kind="Internal").ap()
    qw_d = nc.dram_tensor("qw8", [C, C], F8, kind="ExternalInput").ap()   # (qkv_w[0:512].T * 256)
    kw_d = nc.dram_tensor("kw8", [C, C], F8, kind="ExternalInput").ap()
    vw_d = nc.dram_tensor("vw8", [C, C], F8, kind="ExternalInput").ap()
    pw_d = nc.dram_tensor("pw8", [C, C], F8, kind="ExternalInput").ap()   # proj_w.T * 256
    bias_d = nc.dram_tensor("biases", [4, C], F32, kind="ExternalInput").ap()  # qb,kb,vb,pb
    nwb_d = nc.dram_tensor("nwb", [2, C], F32, kind="ExternalInput").ap()      # norm_w, norm_b
    m8_d = nc.dram_tensor("mask8", [128, 8], F32, kind="ExternalInput").ap()
    m8t_d = nc.dram_tensor("mask8t", [8, 128], F32, kind="ExternalInput").ap()
    cs_dram = nc.dram_tensor("cs_scratch", [8, IHALF], F32, kind="Internal").ap()
    out_d = nc.dram_tensor("out", [C, IHALF], F32, kind="ExternalOutput").ap()
    dbg = {}
    if DBG:
        dbg["h"] = nc.dram_tensor("dbg_h", [128, CT, S], F8, kind="ExternalOutput").ap()
        dbg["k"] = nc.dram_tensor("dbg_k", [128, CT, S], BF16, kind="ExternalOutput").ap()
        dbg["q"] = nc.dram_tensor("dbg_q", [128, CT, IHALF], BF16, kind="ExternalOutput").ap()
        dbg["vt"] = nc.dram_tensor("dbg_vt", [128, ST, HEADS, 66], F8, kind="ExternalOutput").ap()
        dbg["ex"] = nc.dram_tensor("dbg_ex", [128, 2, 2, 2, IWS], F8, kind="ExternalOutput").ap()
        dbg["on"] = nc.dram_tensor("dbg_on", [128, CT, IHALF], F8, kind="ExternalOutput").ap()
        dbg["cs"] = nc.dram_tensor("dbg_cs", [4, 2, IHALF], F32, kind="ExternalOutput").ap()

    with tile.TileContext(nc) as tc:
        with ExitStack() as ctx:
            const = ctx.enter_context(tc.tile_pool(name="const", bufs=1))

            wts = {}
            for nm, ap in [("qw", qw_d), ("kw", kw_d), ("vw", vw_d), ("pw", pw_d)]:
                t8 = const.tile([128, CT, C], F8, name=f"{nm}_f8")
                nc.sync.dma_start(out=t8, in_=ap.rearrange("(kt p) c -> p kt c", p=128))
                wts[nm] = t8

            bias_cols = const.tile([128, 4, CT], F32)
            nc.sync.dma_start(out=bias_cols,
                              in_=bias_d.rearrange("w (kt p) -> p w kt", p=128))
            nwb_t = const.tile([128, 2, CT], F32)
            nc.sync.dma_start(out=nwb_t, in_=nwb_d.rearrange("w (kt p) -> p w kt", p=128))

            wn_t = const.tile([GROUPS, 4, CT, 128], BF16)
            nc.sync.dma_start(out=wn_t,
                              in_=wn_d.rearrange("w g (mt p) -> g w mt p", p=128))

            m8_t = const.tile([128, 8], F32)
            nc.sync.dma_start(out=m8_t, in_=m8_d)
            m8t_t = const.tile([8, 128], F32)
            nc.sync.dma_start(out=m8t_t, in_=m8t_d)

            eps_t = const.tile([8, 1], F32)
            nc.vector.memset(eps_t, EPS)

            consts = (wts, wn_t, bias_cols, nwb_t, m8_t, m8t_t, eps_t)
            if reps == 1:
                _one_pass(nc, tc, x_d, x8_d, out_d, cs_dram, beta_dram, consts, dbg)
            else:
                with tc.For_i(0, reps, 1):
                    _one_pass(nc, tc, x_d, x8_d, out_d, cs_dram, beta_dram, consts, dbg)
    nc.compile()
    return nc


def _one_pass(nc, tc, x_d, x8_d, out_d, cs_dram, beta_dram, consts, dbg={}):
    (wts, wn_t, bias_cols, nwb_t, m8_t, m8t_t, eps_t) = consts
    with ExitStack() as ps:
        big = ps.enter_context(tc.tile_pool(name="big", bufs=1))
        x8_all = big.tile([128, CT, S], F8)             # 16 KB (host fp8 cast)
        wsc = {nm: big.tile([128, CT, C], F8, name=f"{nm}_sc")
               for nm in ("qw", "kw", "vw")}            # affine-scaled weights
        bias_eff = big.tile([128, 4, CT], F32)          # q/k/v/proj biases after absorb
        k_all = big.tile([128, CT, S], BF16)            # 32 KB
        q_all = big.tile([128, CT, IHALF], BF16)        # 16 KB
        vt_ext = big.tile([128, ST, HEADS, 66], F8)     # ~16.5 KB
        o_raw8 = big.tile([128, CT, IHALF], F8)         # 8 KB  (unnormalized o/16)
        o_norm = big.tile([128, CT, IHALF], F8)         # 8 KB
        ex_all = big.tile([128, 2, 2, 2, IWS], F8)      # (head, jtpbuf, t, cols) 2 KB

        svec = ps.enter_context(tc.tile_pool(name="svec", bufs=1))
        aff_s = svec.tile([128, CT], F32)
        aff_t = svec.tile([128, CT], F32)
        # colsums/16 for (hp, p) at partition 32*hp, free dim p (32-aligned
        # partition bases are required for engine operands); the reciprocal
        # runs in place per head-pair.
        cs_ab = svec.tile([128, 2, IHALF], F32)

        # =============== P1: x8 load + groupnorm statistics ===============
        # Stats run on the host-cast fp8 x (stat noise ~1e-3 rel, well inside
        # budget) so the 8MB f32 x never enters SBUF.  The residual is
        # instead pre-copied DRAM->DRAM into out and the proj result is
        # DMA-accumulated onto it at the end.
        for t in range(CT):
            eng = nc.sync if t % 2 == 0 else nc.scalar
            eng.dma_start(out=x8_all[:, t, :], in_=x8_d[t * 128:(t + 1) * 128, :])

        with tc.tile_pool(name="stat", bufs=2) as stat, \
             tc.tile_pool(name="gn_ps", bufs=1, space="PSUM") as gn_ps:
            dummy_ln = stat.tile([8, 1], F32, tag="dln", bufs=1)
            nc.scalar.activation(dummy_ln, eps_t, LN)
            gstats = gn_ps.tile([8, CT, 2], F32)
            warm_ps = gn_ps.tile([6, 512], F32)
            warm_src = stat.tile([128, 6], F8, tag="wsrc", bufs=1)
            for t in range(CT):
                bnst = stat.tile([128, 8, 6], F32, tag="bnst")
                for sg in range(8):
                    nc.vector.bn_stats(out=bnst[:, sg, :],
                                       in_=x8_all[:, t, sg * 512:(sg + 1) * 512])
                mv = stat.tile([128, 2], F32, tag="mv")
                nc.vector.bn_aggr(out=mv, in_=bnst)
                st_t = stat.tile([128, 2], F32, tag="st")   # (mean, 2nd moment)
                nc.vector.tensor_copy(st_t[:, 0:1], mv[:, 0:1])
                sq = stat.tile([128, 1], F32, tag="sq")
                nc.vector.tensor_mul(sq, mv[:, 0:1], mv[:, 0:1])
                nc.vector.tensor_add(st_t[:, 1:2], mv[:, 1:2], sq)
                nc.tensor.matmul(gstats[:, t, :], m8_t, st_t, start=True, stop=True)
                if t == 0:
                    nc.vector.tensor_copy(warm_src, bnst[:, 0, :])
            # PE keep-warm dummies after the gstats chain (bridge to qkv)
            for _ in range(2):
                nc.tensor.matmul(warm_ps, warm_src, x8_all[:, 0, 0:512],
                                 start=True, stop=True)

            gs = stat.tile([8, CT, 2], F32, tag="gs")
            nc.vector.tensor_copy(gs, gstats)
            grp = stat.tile([8, CT, 2], F32, tag="grp")   # (mu_g, rstd_g)
            var_all = stat.tile([8, CT], F32, tag="var")
            for t in range(CT):
                nc.scalar.mul(grp[:, t, 0:1], gs[:, t, 0:1], 1.0 / GSIZE)
                e_t = stat.tile([8, 1], F32, tag="e")
                nc.scalar.mul(e_t, gs[:, t, 1:2], 1.0 / GSIZE)
                musq = stat.tile([8, 1], F32, tag="musq")
                nc.vector.tensor_mul(musq, grp[:, t, 0:1], grp[:, t, 0:1])
                nc.vector.tensor_sub(var_all[:, t:t + 1], e_t, musq)
            # rstd = exp(-0.5*ln(var+eps)), batched over all CT tiles:
            # one Ln + one Exp = exactly two ACT table loads, in the prologue.
            lnv = stat.tile([8, CT], F32, tag="lnv")
            nc.scalar.activation(lnv, var_all, LN, bias=eps_t)
            nc.scalar.activation(grp[:, :, 1], lnv, EXP, scale=-0.5)

            bc_ps = gn_ps.tile([128, CT, 2], F32)
            for t in range(CT):
                nc.tensor.matmul(bc_ps[:, t, :], m8t_t, grp[:, t, :], start=True, stop=True)
            bc = stat.tile([128, CT, 2], F32, tag="bc")
            nc.vector.tensor_copy(bc, bc_ps)
            nc.vector.tensor_mul(aff_s, nwb_t[:, 0, :], bc[:, :, 1])
            # beta_g = mu_g * rstd_g -> [32, 1] via DRAM roundtrip (engines
            # cannot re-map partitions; DMA can)
            beta8 = stat.tile([8, CT], BF16, tag="beta8")
            nc.vector.tensor_mul(beta8, grp[:, :, 0], grp[:, :, 1])
            wb = nc.sync.dma_start(out=beta_dram.rearrange("(kt i) o -> i (kt o)", i=8),
                                   in_=beta8)
            tc.chain_iter_dep("betadram", wb.ins)
            beta32 = stat.tile([GROUPS, 1], BF16, tag="beta32")
            rb_ = nc.sync.dma_start(out=beta32, in_=beta_dram)
            tc.chain_iter_dep("betadram", rb_.ins)

        # =============== P2: absorb groupnorm affine into qkv weights ======
        # w'[o,c] = w[o,c] * s[c] (per-partition scale of the stationary
        # side); bias' = b + W@norm_b (host) - Wn @ beta (12 tiny f32 MMs).
        for i, nm in enumerate(("qw", "kw", "vw")):
            for kt in range(CT):
                if (i + kt) % 2 == 0:
                    nc.scalar.activation(wsc[nm][:, kt, :], wts[nm][:, kt, :],
                                         IDENT, scale=aff_s[:, kt:kt + 1])
                else:
                    nc.vector.tensor_scalar(
                        out=wsc[nm][:, kt, :], in0=wts[nm][:, kt, :],
                        scalar1=aff_s[:, kt:kt + 1], scalar2=None, op0=MULT)
        with tc.tile_pool(name="bias_ps", bufs=1, space="PSUM") as bias_ps:
            bps = bias_ps.tile([128, 4, CT], F32)
            for w in range(4):
                for mt in range(CT):
                    nc.tensor.matmul(bps[:, w, mt:mt + 1], wn_t[:, w, mt, :],
                                     beta32, start=True, stop=True)
            nc.vector.tensor_sub(bias_eff, bias_cols, bps)

        nc.vector.memset(vt_ext[:, :, :, 64:66], 0.0)
        nc.vector.memset(vt_ext[:, :, :, 64:65], 1.0)

        # =============== P3: qkv projections (fp8 DoubleRow) ===============
        with tc.tile_pool(name="qkv_ps", bufs=3, space="PSUM") as qkv_ps:
            for dst, w8, bidx, ncols in ((k_all, wsc["kw"], 1, S),
                                         (q_all, wsc["qw"], 0, IHALF)):
                for mt in range(CT):
                    for ic2 in range(ncols // 1024):
                        acc = qkv_ps.tile([128, 1024], F32, tag="qkps")
                        for j in range(2):
                            icol = 2 * ic2 + j
                            for kp in range(2):
                                nc.tensor.matmul(
                                    acc[:, j * 512:(j + 1) * 512],
                                    w8[:, 2 * kp:2 * kp + 2, mt * 128:(mt + 1) * 128],
                                    x8_all[:, 2 * kp:2 * kp + 2, icol * 512:(icol + 1) * 512],
                                    start=(kp == 0), stop=(kp == 1), perf_mode=DR)
                        # one 1024-wide evac per pair, split ACT/DVE
                        if (mt + ic2) % 2 == 0:
                            nc.scalar.activation(
                                dst[:, mt, ic2 * 1024:(ic2 + 1) * 1024], acc, IDENT,
                                scale=1.0 / WSCALE, bias=bias_eff[:, bidx, mt:mt + 1])
                        else:
                            nc.vector.tensor_scalar(
                                out=dst[:, mt, ic2 * 1024:(ic2 + 1) * 1024], in0=acc,
                                scalar1=1.0 / WSCALE, scalar2=bias_eff[:, bidx, mt:mt + 1],
                                op0=MULT, op1=ADD)
            # V^T: [S, 512c] scattered into vt_ext (64 cols per head + ones
            # col).  The V bias is folded into the proj bias on the host
            # (pb' = pb + proj_w @ vb), so the evac is a pure scale and can
            # split across ACT and DVE.
            for sp in range(ST // 2):
                acc = qkv_ps.tile([128, 1024], F32, tag="qkps")
                for j in range(2):
                    st = 2 * sp + j
                    for kp in range(2):
                        nc.tensor.matmul(
                            acc[:, j * 512:(j + 1) * 512],
                            x8_all[:, 2 * kp:2 * kp + 2, st * 128:(st + 1) * 128],
                            wsc["vw"][:, 2 * kp:2 * kp + 2, :],
                            start=(kp == 0), stop=(kp == 1), perf_mode=DR)
                if sp % 2 == 0:
                    nc.scalar.activation(
                        vt_ext[:, 2 * sp:2 * sp + 2, :, 0:64],
                        acc.rearrange("p (s h d) -> p s h d", s=2, d=HD), IDENT,
                        scale=1.0 / WSCALE)
                else:
                    nc.vector.tensor_scalar(
                        out=vt_ext[:, 2 * sp:2 * sp + 2, :, 0:64],
                        in0=acc.rearrange("p (s h d) -> p s h d", s=2, d=HD),
                        scalar1=1.0 / WSCALE, scalar2=None, op0=MULT)

        if dbg:
            nc.sync.dma_start(out=dbg["h"], in_=x8_all)
            nc.sync.dma_start(out=dbg["k"], in_=k_all)
            nc.sync.dma_start(out=dbg["q"], in_=q_all)
            nc.sync.dma_start(out=dbg["vt"], in_=vt_ext)

        # residual precopy: out <- x (runs during attention; bus is idle)
        xc = nc.scalar.dma_start(out=out_d, in_=x_d[:, 0:IHALF])
        for mt in range(CT):
            tc.chain_iter_dep(f"xres{mt}", xc.ins)

        # =============== P4: attention ===============
        # Per (head-pair hp, query block iw of 512): scores bf16 with 2-head
        # row packing -> PSUM [128, 2, 512]; exp on ScalarE/DVE per EXP_PAT
        # -> ex fp8; O accumulates per head in fp8 DoubleRow over jt pairs,
        # pipelined one jt-pair behind the scores/exp.  o_ps double-buffered
        # across blocks; evacuation of block n is issued early in block n+1.
        with tc.tile_pool(name="att_ps", bufs=1, space="PSUM") as att_ps, \
             tc.tile_pool(name="att_sb", bufs=1) as att_sb:
            o_ps = [att_ps.tile([65, IWS], F32, tag=f"o{p}", name=f"o_ps{p}")
                    for p in range(2)]
            rb_t = att_sb.tile([128, IHALF], F32, name="rb")

            blocks = [(hp, iw) for hp in range(CT) for iw in range(NIW)]
            deferred = {"A": [], "D": []}   # evac closures from previous block
            chain = {}                      # jt -> closure (recip / dma / mult)

            def mk_evac(hp, iw, o_ps):
                # evacuate unnormalized o (x 1/16) + gather colsum rows
                def evac_a():
                    nc.scalar.activation(
                        o_raw8[0:64, hp, iw * IWS:(iw + 1) * IWS],
                        o_ps[0][0:64, :], IDENT, scale=OSCALE)
                    nc.scalar.activation(
                        cs_ab[32 * hp:32 * hp + 1, 0, iw * IWS:(iw + 1) * IWS],
                        o_ps[0][64:65, :], IDENT, scale=OSCALE)
                def evac_d():
                    nc.vector.tensor_scalar(
                        out=o_raw8[64:128, hp, iw * IWS:(iw + 1) * IWS],
                        in0=o_ps[1][0:64, :], scalar1=OSCALE, scalar2=None, op0=MULT)
                    nc.vector.tensor_scalar(
                        out=cs_ab[32 * hp:32 * hp + 1, 1, iw * IWS:(iw + 1) * IWS],
                        in0=o_ps[1][64:65, :], scalar1=OSCALE, scalar2=None, op0=MULT)
                return evac_a, evac_d

            def mk_chain(hp, rb):
                # 16/cs for head-pair hp: DRAM-roundtrip broadcast of the raw
                # colsums to 128 partitions, then one in-place DVE
                # reciprocal_approx_fast on the broadcast tile (base partition
                # 0 -- the custom-DVE op mis-addresses nonzero base
                # partitions; engine cost is free-size only, so recip on
                # [128, 2048] costs the same as on [1, 2048]), then one
                # batched normalize multiply.
                def c_dma():
                    w = nc.sync.dma_start(out=cs_dram[2 * hp:2 * hp + 2, :],
                                          in_=cs_ab[32 * hp:32 * hp + 1, :, :])
                    tc.chain_iter_dep("csdram", w.ins)
                    b0 = nc.sync.dma_start(
                        out=rb[0:64, :],
                        in_=cs_dram[2 * hp:2 * hp + 1, :].to_broadcast([64, IHALF]))
                    tc.chain_iter_dep("csdram", b0.ins)
                    b1 = nc.sync.dma_start(
                        out=rb[64:128, :],
                        in_=cs_dram[2 * hp + 1:2 * hp + 2, :].to_broadcast([64, IHALF]))
                    tc.chain_iter_dep("csdram", b1.ins)
                def c_recip():
                    nc.vector.reciprocal_approx_fast(out=rb, in_=rb)
                def c_mult():
                    # split the batched normalize between DVE and GPSIMD
                    nc.vector.tensor_mul(o_norm[:, hp, 0:IHALF // 2],
                                         o_raw8[:, hp, 0:IHALF // 2],
                                         rb[:, 0:IHALF // 2])
                    nc.gpsimd.tensor_mul(o_norm[:, hp, IHALF // 2:],
                                         o_raw8[:, hp, IHALF // 2:],
                                         rb[:, IHALF // 2:])
                return [c_dma, c_recip, c_mult]

            for bi, (hp, iw) in enumerate(blocks):
                def emit_o(jtp, p):
                    nc.tensor.matmul(
                        o_ps[p],
                        vt_ext[:, 2 * jtp:2 * jtp + 2, 2 * hp + p, 0:65],
                        ex_all[:, p, jtp % 2, :, :],
                        start=(jtp == 0), stop=(jtp == ST // 2 - 1),
                        perf_mode=DR, skip_group_check=True)

                for jtp in range(ST // 2):
                    for t in range(2):
                        jt = 2 * jtp + t
                        sc = att_ps.tile([128, 2, IWS], F32, tag=f"sc{jt % 3}",
                                         name=f"sc{jt % 3}")
                        for p in range(2):
                            nc.tensor.matmul(
                                sc[:, p, :],
                                k_all[64 * p:64 * p + 64, hp, jt * 128:(jt + 1) * 128],
                                q_all[64 * p:64 * p + 64, hp, iw * IWS:(iw + 1) * IWS],
                                start=True, stop=True)
                        ex_ap = ex_all[:, :, jtp % 2, t, :]
                        if EXP_PAT[jt] == "A":
                            nc.scalar.activation(ex_ap, sc, EXP, scale=SCALE)
                        else:
                            nc.vector.tensor_scalar(
                                out=ex_ap.bitcast(I8), in0=sc,
                                scalar1=A8, scalar2=B8, op0=MULT, op1=ADD)
                        # flush previous block's evacuations / recip chain
                        if jt == 1 and deferred["A"]:
                            for f in deferred["A"]:
                                f()
                            deferred["A"] = []
                        elif jt == 2 and deferred["D"]:
                            for f in deferred["D"]:
                                f()
                            deferred["D"] = []
                        elif jt in chain:
                            chain.pop(jt)()
                    if jtp > 0:
                        for p in range(2):
                            emit_o(jtp - 1, p)
                    if dbg and hp == 0 and iw == 0 and jtp == 0:
                        nc.sync.dma_start(out=dbg["ex"], in_=ex_all)
                for p in range(2):
                    emit_o(ST // 2 - 1, p)

                evac_a, evac_d = mk_evac(hp, iw, o_ps)
                if bi == len(blocks) - 1:
                    evac_a(); evac_d()
                    for f in mk_chain(hp, rb_t):
                        f()
                else:
                    deferred["A"].append(evac_a)
                    deferred["D"].append(evac_d)
                    if iw == NIW - 1:
                        # schedule hp's normalize chain into upcoming blocks
                        fns = mk_chain(hp, rb_t)
                        chain[6] = fns[0]    # dma    (block bi+1, jt 6)
                        chain[18] = fns[1]   # recip  (block bi+1, jt 18)
                        chain[20] = fns[2]   # mult   (block bi+1, jt 20)


        if dbg:
            nc.sync.dma_start(out=dbg["on"], in_=o_norm)
            for hp in range(CT):
                nc.sync.dma_start(out=dbg["cs"][hp], in_=cs_ab[32 * hp:32 * hp + 1, :, :])

        # =============== P5: proj + residual ===============
        # out already holds x (DRAM precopy); evac applies scale+bias, then a
        # gpsimd SWDGE DMA accumulates the proj result onto out (one per mt).
        with tc.tile_pool(name="pj_ps", bufs=3, space="PSUM") as pj_ps, \
             tc.tile_pool(name="pj_sb", bufs=2) as pj_sb:
            for mt in range(CT):
                o_mt = pj_sb.tile([128, IHALF], F32, tag="ot")
                for icol in range(IHALF // 512):
                    acc = pj_ps.tile([128, 512], F32, tag="pj")
                    for kp in range(2):
                        nc.tensor.matmul(
                            acc, wts["pw"][:, 2 * kp:2 * kp + 2, mt * 128:(mt + 1) * 128],
                            o_norm[:, 2 * kp:2 * kp + 2, icol * 512:(icol + 1) * 512],
                            start=(kp == 0), stop=(kp == 1), perf_mode=DR)
                    if icol % 2 == 0:
                        nc.scalar.activation(
                            o_mt[:, icol * 512:(icol + 1) * 512], acc, IDENT,
                            scale=1.0 / WSCALE, bias=bias_eff[:, 3, mt:mt + 1])
                    else:
                        nc.vector.tensor_scalar(
                            out=o_mt[:, icol * 512:(icol + 1) * 512], in0=acc,
                            scalar1=1.0 / WSCALE, scalar2=bias_eff[:, 3, mt:mt + 1],
                            op0=MULT, op1=ADD)
                ac = nc.gpsimd.dma_start(
                    out=out_d[mt * 128:(mt + 1) * 128, :], in_=o_mt,
                    accum_op=ADD)
                tc.chain_iter_dep(f"xres{mt}", ac.ins)


# ---------------------------------------------------------------------------
# host side
# ---------------------------------------------------------------------------

_CACHE = {}


def _get_nc(reps=1):
    if reps not in _CACHE:
        _CACHE[reps] = build_kernel(reps)
    return _CACHE[reps]


def _w8(a):
    return np.ascontiguousarray((a * WSCALE).astype(ml_dtypes.float8_e4m3))


def _make_in_maps(inputs):
    x = np.ascontiguousarray(np.asarray(inputs["x"], dtype=np.float32))
    qkv_w = np.asarray(inputs["qkv_w"], dtype=np.float32)
    qkv_b = np.asarray(inputs["qkv_b"], dtype=np.float32)
    proj_w = np.asarray(inputs["proj_w"], dtype=np.float32)
    proj_b = np.asarray(inputs["proj_b"], dtype=np.float32)
    norm_w = np.asarray(inputs["norm_w"], dtype=np.float32)
    norm_b = np.asarray(inputs["norm_b"], dtype=np.float32)

    wq, wk, wv = qkv_w[0:C], qkv_w[C:2 * C], qkv_w[2 * C:3 * C]
    # group-collapsed weights (for the on-device bias correction -Wn@beta)
    wn3 = [(w * norm_w[None, :]).reshape(C, GROUPS, GSIZE).sum(axis=2)
           for w in (wq, wk, wv)]                      # each [512(out), 32(g)]
    wn = np.stack([wn3[0].T, wn3[1].T, wn3[2].T,
                   (proj_w @ wn3[2]).T])               # [4, 32, 512]
    vb_full = qkv_b[2 * C:3 * C] + wv @ norm_b   # v-bias base (beta part on dev)
    shared = {
        "qw8": _w8(wq.T),
        "kw8": _w8(wk.T),
        "vw8": _w8(wv.T),
        "pw8": _w8(proj_w.T),
        "wn": np.ascontiguousarray(wn.astype(ml_dtypes.bfloat16)),
        "biases": np.ascontiguousarray(
            np.stack([qkv_b[0:C] + wq @ norm_b, qkv_b[C:2 * C] + wk @ norm_b,
                      vb_full,
                      proj_b + proj_w @ vb_full])),
        "nwb": np.ascontiguousarray(np.stack([norm_w, norm_b])),
        "mask8": np.ascontiguousarray(
            (np.arange(128)[:, None] // GSIZE == np.arange(8)[None, :]).astype(np.float32)),
        "mask8t": np.ascontiguousarray(
            (np.arange(128)[None, :] // GSIZE == np.arange(8)[:, None]).astype(np.float32)),
    }

    in_maps = []
    for core in range(8):
        b, ih = core // 2, core % 2
        xb = x[b].reshape(C, S)
        if ih == 1:
            xb = np.concatenate([xb[:, IHALF:], xb[:, :IHALF]], axis=1)
        m = dict(shared)
        m["x"] = np.ascontiguousarray(xb)
        m["x8"] = np.ascontiguousarray(xb.astype(ml_dtypes.float8_e4m3))
        in_maps.append(m)
    return in_maps


def kernel(**inputs):
    nc = _get_nc(1)
    in_maps = _make_in_maps(inputs)
    res = run_bass_kernel_spmd(nc, in_maps, core_ids=list(range(8)))
    y = np.empty((B, C, S), dtype=np.float32)
    for core in range(8):
        b, ih = core // 2, core % 2
        y[b][:, ih * IHALF:(ih + 1) * IHALF] = res.results[core]["out"]
    return y.reshape(B, C, HH, WW)
